# revision 28
# baseline (speedup 1.0000x reference)
"""Block-sparse attention (sliding window of 3 x 64-token blocks) on 8 trn2 cores.

Problem: B=1, H=16, S=4096, D=128, fp32 I/O. Token i attends to token j iff
|i//64 - j//64| <= 1, i.e. a 192-key window per 64-query block.

Sharding: head-parallel - 2 heads per NeuronCore, no cross-core traffic.

The kernel is DMA-bandwidth-bound (per-core HBM ~358 GB/s): inputs must stay
fp16 (fp8 fails the 2e-2 error budget empirically), so per-core traffic is
6.44 MB in + 2.11 MB out (outputs in fp16, halved vs fp32) ~= 24 us at peak.
Everything else is shaped to hide under that stream:

Per-core kernel (per head = 32 q-tiles of 128, processed as 8 groups of
4 tiles / 512 queries):
  - Host packs ONE fp16 input tensor per head in consumption order: 9 chunks
    (chunk 0 split at pair granularity for a faster first MM1), each
    [qT | kT | va] where qT = Q^T [128d, S], kT = K^T padded by 64 keys
    on each end [128d, 4224], va = V augmented with a ones-column rearranged
    to [128, 33*129] (128-key chunk c at cols [129c, 129c+129)); the
    ones-column is zero on the pad rows so pad keys self-neutralize.
  - MM1 (PE), 3 matmuls per 128-q pair instead of 4: the interior key chunk
    serves both q-tiles of the pair with one N=256 matmul. Scores land as
    [keys=128, q] in a 2-bank PSUM tile [128, 1024] per group (2 pairs).
  - ACT: ONE 1024-col exp per group (PSUM reads may span banks) -> fp16 P.
    No max-subtraction: scores ~N(0,1), exact softmax up to rounding.
  - GPSIMD: 2 strided-AP memsets per group zero the four disallowed 64x64
    corners of P post-exp in SBUF (gpsimd has no PSUM port).
  - MM2 (PE): per tile, psO[q=128, 129] accumulates P_A^T.T @ VA_A +
    P_B^T.T @ VB; col 128 (ones-column) gives the softmax denominator free.
  - DVE: copy psO -> fp16 SBUF (normalization division happens on HOST:
    out = PV/den).
  - Output written as [128, 16 pairs * 258] fp16; host divides + reassembles.

Emission is software-pipelined (group n+1's MM1 before group n's tail) and
all input DMAs are emitted first so they outrank output DMAs in scheduler
priority. PE warmup matmuls run inside the pre-data window to engage the HAM
clock gate (1.2 -> 2.4 GHz) as early as possible; everything before the flip
runs PE-throughput-bound at the cold clock.
"""

import bisect
import math

import numpy as np

B, H, S, D = 1, 16, 4096, 128
N_CORES = 8
HPC = H // N_CORES          # heads per core
TILE = 128
NT = S // TILE              # 32 query tiles per head
NPAIR = NT // 2             # 16 pairs (2 tiles each)
NGRP = NPAIR // 2           # 8 groups (2 pairs each)
PAD = 64
SPAD = S + 2 * PAD          # 4224 padded keys
NCHUNK = SPAD // TILE       # 33 key chunks
VAW = NCHUNK * (D + 1)      # 4257 cols of rearranged augmented V
SCALE = 1.0 / math.sqrt(D)

# Packed-input chunking: one chunk per group, consumption-aligned so every
# kernel slice stays inside one segment. Chunk 0 is split at pair-0
# granularity (qt 256 / kt 384 / va 258): the first MM1 only gates on the
# ~230KB half-chunk, starting compute ~1.2us sooner than the full 460KB
# chunk would.
QT_B = [0, 256] + [512 * g for g in range(1, NGRP + 1)]        # 0,256,512,...,4096
KT_B = [0, 384] + [512 * g + 640 for g in range(NGRP - 1)] + [SPAD]
VA_B = [0, 258] + [516 * g + 645 for g in range(NGRP - 1)] + [VAW]
NCK = NGRP + 1
QT_W = [QT_B[i + 1] - QT_B[i] for i in range(NCK)]
KT_W = [KT_B[i + 1] - KT_B[i] for i in range(NCK)]
VA_W = [VA_B[i + 1] - VA_B[i] for i in range(NCK)]
CHUNK_W = [QT_W[i] + KT_W[i] + VA_W[i] for i in range(NCK)]
BASE = [0]
for i in range(NCK):
    BASE.append(BASE[-1] + CHUNK_W[i])
W_PACK = BASE[-1]
OUTW = NPAIR * 258          # 4128 fp16 cols per head

_PROGRAM = None


def _qt_off(x):
    i = bisect.bisect_right(QT_B, x) - 1
    return BASE[i] + (x - QT_B[i]), i


def _kt_off(y):
    i = bisect.bisect_right(KT_B, y) - 1
    return BASE[i] + QT_W[i] + (y - KT_B[i]), i


def _va_off(z):
    i = bisect.bisect_right(VA_B, z) - 1
    return BASE[i] + QT_W[i] + KT_W[i] + (z - VA_B[i]), i


def _build_program():
    from contextlib import ExitStack

    import concourse.mybir as mybir
    import concourse.tile as tile
    from concourse import bacc

    f16 = mybir.dt.float16
    f32 = mybir.dt.float32
    Exp = mybir.ActivationFunctionType.Exp

    nc = bacc.Bacc("TRN2", target_bir_lowering=False, debug=False)
    qkv_d = nc.declare_dram_parameter("qkv", [HPC, 128, W_PACK], f16, isOutput=False)
    out_d = nc.declare_dram_parameter("out", [HPC, 128, OUTW], f16, isOutput=True)

    def qt_sl(sb, x0, w):
        off, i = _qt_off(x0)
        assert x0 + w <= QT_B[i + 1], (x0, w)
        return sb[:, off:off + w]

    def kt_sl(sb, y0, w):
        off, i = _kt_off(y0)
        assert y0 + w <= KT_B[i + 1], (y0, w)
        return sb[:, off:off + w]

    def va_sl(sb, z0, w):
        off, i = _va_off(z0)
        assert z0 + w <= VA_B[i + 1], (z0, w)
        return sb[:, off:off + w]

    with tile.TileContext(nc) as tc, ExitStack() as ctx:
        io_pool = ctx.enter_context(tc.tile_pool(name="io", bufs=2))
        out_pool = ctx.enter_context(tc.tile_pool(name="outp", bufs=2))
        p_pool = ctx.enter_context(tc.tile_pool(name="p", bufs=3))
        # PSUM budget (8 banks): ps 2 bufs x 2 banks + po 2 bufs x 2 banks.
        # po=2 is the load-bearing choice: with po=1 the steady state locks
        # to a metronomic 1.50us loop [MM2span 670 + prop + CAST 694 + prop]
        # because MM2(n) WAR-waits CAST(n-1) draining the single po buffer.
        # ps=2's own loop (ACT(n) <- MM1(n) <- ps-WAR <- ACT(n-2)) is only
        # ~950ns/group at full clock with the dense 3-matmul MM1, safely
        # under the ACT engine's 1.11us/group exp throughput.
        ps_pool = ctx.enter_context(tc.tile_pool(name="ps", bufs=2, space="PSUM"))
        po_pool = ctx.enter_context(tc.tile_pool(name="po", bufs=2, space="PSUM"))

        # PE warmup: the HAM clock gate needs one ~3.4us window of sustained
        # PE activity to flip 1.2 -> 2.4 GHz. Without fillers it flips only
        # mid-kernel (measured 17-26us in) and every matmul before that runs
        # at half rate, putting PE above the chunk-arrival pace. Fill the
        # pre-data window with N=512 dummy matmuls so the flip lands at
        # ~4.5us, just as real compute ramps.
        # warm is mostly uninitialized: a full memset would delay the first
        # filler by ~1us of GPSIMD queue time; garbage operands are harmless
        # (the filler outputs are never read). The 1-element DVE memset just
        # satisfies the tile allocator's written-before-read requirement.
        warm_pool = ctx.enter_context(tc.tile_pool(name="warm", bufs=1))
        warm = warm_pool.tile([128, 512], f16, tag="warm")
        nc.vector.memset(warm[0:1, 0:1], 0.0)
        ps_warm = ps_pool.tile([128, 1024], f32, tag="ps", name="ps_warm")
        # 8x448-col fillers (~450ns issue cadence at the ~1GHz cold clock)
        # bridge the PE queue's preamble end (~7.5us) to first-chunk arrival
        # (~10.5us) with NO idle gap, so the HAM activity window that flips
        # the clock gate 1.2->2.4GHz starts counting from ~7.5us.
        for _ in range(8):
            nc.tensor.matmul(
                ps_warm[0:1, 0:448], lhsT=warm[:, 0:1], rhs=warm[:, 0:448],
                start=True, stop=True,
            )
        # Dummy exp so walrus schedules the ACT table load during the
        # pre-data window rather than before the first real activation.
        # Output goes to a separate tile so it doesn't WAR against the
        # fillers reading warm.
        dummy = warm_pool.tile([1, 16], f16, tag="dummy")
        nc.scalar.activation(dummy[0:1, 0:8], warm[0:1, 8:16], Exp, bias=0.0,
                             scale=1.0)



        # Load phase: ALL input DMAs (both heads) emitted first so they
        # outrank output DMAs in scheduler priority. Multi-wait needs on the
        # consuming matmuls are handled by framework-inserted event-
        # semaphore splits on the PE queue (~100ns each).
        io_sbs = []
        for h in range(HPC):
            io_sb = io_pool.tile([128, W_PACK], f16, tag="io")
            io_sbs.append(io_sb)
        for h in range(HPC):
            io_sb = io_sbs[h]
            for c in range(NCK):
                nc.sync.dma_start(
                    io_sb[:, BASE[c]:BASE[c + 1]], qkv_d[h, :, BASE[c]:BASE[c + 1]]
                )

        groups = [(h, g) for h in range(HPC) for g in range(NGRP)]
        out_sbs = {}
        ps_tiles = {}

        def emit_mm1(h, g):
            # Dense 3-matmul MM1 per pair (the interior key chunk serves
            # both q-tiles with one N=256 matmul). NOTE: partition-offset
            # sub-matmuls that skip the dead corners were tried and are a
            # big LOSS - writing PSUM partitions 64:128 switches the PE
            # column group (col_grp h0<->h64), ~190ns per matmul instead of
            # 56, and the HAM clock gate never engaged full rate.
            io_sb = io_sbs[h]
            ps = ps_pool.tile([128, 1024], f32, tag="ps")
            ps_tiles[(h, g)] = ps
            for j in range(2):           # pairs 2g, 2g+1
                u = 2 * g + j
                c0 = 512 * j
                # Pair u covers q-tiles 2u, 2u+1; padded key window
                # [256u, 256u+384) = key chunks u*2 .. u*2+2 at 128 stride.
                nc.tensor.matmul(
                    ps[:, c0:c0 + 128],
                    lhsT=kt_sl(io_sb, 256 * u, 128),
                    rhs=qt_sl(io_sb, 256 * u, 128), start=True, stop=True,
                )
                nc.tensor.matmul(
                    ps[:, c0 + 128:c0 + 384],
                    lhsT=kt_sl(io_sb, 256 * u + 128, 128),
                    rhs=qt_sl(io_sb, 256 * u, 256), start=True, stop=True,
                )
                nc.tensor.matmul(
                    ps[:, c0 + 384:c0 + 512],
                    lhsT=kt_sl(io_sb, 256 * u + 256, 128),
                    rhs=qt_sl(io_sb, 256 * u + 128, 128), start=True, stop=True,
                )



        def emit_tail(h, g):
            io_sb = io_sbs[h]
            out_sb = out_sbs[h]
            ps = ps_tiles.pop((h, g))
            p_sb = p_pool.tile([128, 1024], f16, tag="p")
            nc.scalar.activation(p_sb[:], ps[:], Exp, bias=0.0, scale=SCALE)
            # Kill the four disallowed 64x64 corners POST-exp on the fp16 P
            # tile in SBUF via the otherwise-idle GPSIMD engine (which has
            # no PSUM port, but P is in SBUF). Keeping kills off the DVE and
            # off the ACT-gating path leaves ACT(n) <- MM1(n) <- ACT(n-2)
            # as the only ps-WAR loop (~950ns/group at full clock). Edge
            # pads need no kill: pad kt columns are zero so scores exp to
            # exactly 1, and packed VA pad rows are all-zero INCLUDING the
            # ones-column, contributing 0 to both PV and the denominator.
            pr = p_sb.rearrange("p (a b) -> p a b", b=256)
            nc.gpsimd.memset(pr[0:64, :, 64:128], 0.0)
            nc.gpsimd.memset(pr[64:128, :, 128:192], 0.0)
            po = po_pool.tile([128, 1024], f32, tag="po")
            for j in range(2):
                u = 2 * g + j
                t0, t1 = 2 * u, 2 * u + 1
                pb = 512 * j
                ob = 512 * j          # pair j's accumulators in bank j
                nc.tensor.matmul(
                    po[:, ob:ob + 129], lhsT=p_sb[:, pb:pb + 128],
                    rhs=va_sl(io_sb, 129 * t0, 129), start=True, stop=False,
                )
                nc.tensor.matmul(
                    po[:, ob:ob + 129], lhsT=p_sb[:, pb + 128:pb + 256],
                    rhs=va_sl(io_sb, 129 * (t0 + 1), 129), start=False, stop=True,
                )
                nc.tensor.matmul(
                    po[:, ob + 129:ob + 258], lhsT=p_sb[:, pb + 256:pb + 384],
                    rhs=va_sl(io_sb, 129 * t1, 129), start=True, stop=False,
                )
                nc.tensor.matmul(
                    po[:, ob + 129:ob + 258], lhsT=p_sb[:, pb + 384:pb + 512],
                    rhs=va_sl(io_sb, 129 * (t1 + 1), 129), start=False, stop=True,
                )
            # ONE strided cast per group moves both pairs' [128,258] blocks
            # (banks 0 and 1 of po) to fp16 SBUF in a single DVE pass.
            src = po.rearrange("p (a b) -> p a b", b=512)[:, :, 0:258]
            dst = out_sb[:, 2 * g * 258:(2 * g + 2) * 258]
            nc.vector.tensor_copy(dst.rearrange("p (a b) -> p a b", b=258), src)
            # Stream output back: 4-pair chunks; the final groups go in
            # smaller chunks so the last DMA (trailing the last pair's
            # compute) is short.
            if g in (1, 3, 5):
                c0, c1 = (g - 1) * 2 * 258, (g + 1) * 2 * 258
                nc.sync.dma_start(out_d[h, :, c0:c1], out_sb[:, c0:c1])
            elif g == 6:
                c0, c1 = 12 * 258, 14 * 258
                nc.sync.dma_start(out_d[h, :, c0:c1], out_sb[:, c0:c1])
            elif g == 7:
                for u in (14, 15):
                    c0, c1 = u * 258, (u + 1) * 258
                    nc.sync.dma_start(out_d[h, :, c0:c1], out_sb[:, c0:c1])

        # DEPTH=2: PE runs MM1(g+2) while group g's ACT->memset->MM2 chain
        # completes, so the memset latency never stalls the PE stream.
        DEPTH = 2
        for n in range(len(groups) + DEPTH):
            if n < len(groups):
                h, g = groups[n]
                if g == 0:
                    out_sb = out_pool.tile([128, OUTW], f16, tag="out")
                    out_sbs[h] = out_sb
                emit_mm1(h, g)
            if n >= DEPTH:
                emit_tail(*groups[n - DEPTH])

    nc.finalize()
    return nc


def _get_program():
    global _PROGRAM
    if _PROGRAM is None:
        _PROGRAM = _build_program()
    return _PROGRAM


def _pack_inputs(q, k, v):
    """q,k,v: [H, S, D] fp32 -> packed [H, 128, W_PACK] fp16 per head."""
    qt = np.ascontiguousarray(q.transpose(0, 2, 1)).astype(np.float16)  # [H,128,S]
    k_pad = np.zeros((H, SPAD, D), np.float32)
    k_pad[:, PAD:PAD + S] = k
    kt = np.ascontiguousarray(k_pad.transpose(0, 2, 1)).astype(np.float16)
    v_aug = np.zeros((H, SPAD, D + 1), np.float32)
    v_aug[:, PAD:PAD + S, :D] = v
    # ones-column only on REAL rows: pad keys then add exp(0)*0 = 0 to both
    # PV and the denominator, so no edge-kill memsets are needed on-device.
    v_aug[:, PAD:PAD + S, D] = 1.0
    va = np.ascontiguousarray(
        v_aug.reshape(H, NCHUNK, 128, D + 1).transpose(0, 2, 1, 3)
    ).reshape(H, 128, VAW).astype(np.float16)
    segs = []
    for c in range(NCK):
        segs.append(qt[:, :, QT_B[c]:QT_B[c + 1]])
        segs.append(kt[:, :, KT_B[c]:KT_B[c + 1]])
        segs.append(va[:, :, VA_B[c]:VA_B[c + 1]])
    return np.ascontiguousarray(np.concatenate(segs, axis=2))


def kernel(q, k, v):
    """q, k, v: [1, 16, 4096, 128] float32 -> [1, 16, 4096, 128] float32."""
    from concourse.bass_utils import run_bass_kernel_spmd

    q = np.asarray(q, dtype=np.float32).reshape(H, S, D)
    k = np.asarray(k, dtype=np.float32).reshape(H, S, D)
    v = np.asarray(v, dtype=np.float32).reshape(H, S, D)

    qkv = _pack_inputs(q, k, v)
    in_maps = [
        {"qkv": np.ascontiguousarray(qkv[c * HPC:(c + 1) * HPC])}
        for c in range(N_CORES)
    ]

    nc = _get_program()
    results = run_bass_kernel_spmd(nc, in_maps, list(range(N_CORES))).results

    out = np.empty((H, S, D), np.float32)
    for c in range(N_CORES):
        o = results[c]["out"]  # [HPC, 128, 16*258] fp16, per tile [PV|den]
        for j in range(HPC):
            x = o[j].astype(np.float32).reshape(128, NT, D + 1)  # [p, t, 129]
            pv = x[:, :, :D] / x[:, :, D:D + 1]     # normalize on host
            out[c * HPC + j] = pv.transpose(1, 0, 2).reshape(S, D)
    return out.reshape(B, H, S, D)



# revision 29
# speedup vs baseline: 1.1347x; 1.1347x over previous
"""Block-sparse attention (sliding window of 3 x 64-token blocks) on 8 trn2 cores.

Problem: B=1, H=16, S=4096, D=128, fp32 I/O. Token i attends to token j iff
|i//64 - j//64| <= 1, i.e. a 192-key window per 64-query block.

Sharding: head-parallel - 2 heads per NeuronCore, no cross-core traffic.

The kernel is DMA-bandwidth-bound (per-core HBM ~358 GB/s): inputs must stay
fp16 (fp8 fails the 2e-2 error budget empirically), so per-core traffic is
6.44 MB in + 2.11 MB out (outputs in fp16, halved vs fp32) ~= 24 us at peak.
Everything else is shaped to hide under that stream:

Per-core kernel (per head = 32 q-tiles of 128, processed as 8 groups of
4 tiles / 512 queries):
  - Host packs ONE fp16 input tensor per head in consumption order: 9 chunks
    (chunk 0 split at pair granularity for a faster first MM1), each
    [qT | kT | va] where qT = Q^T [128d, S], kT = K^T padded by 64 keys
    on each end [128d, 4224], va = V augmented with a ones-column rearranged
    to [128, 33*129] (128-key chunk c at cols [129c, 129c+129)); the
    ones-column is zero on the pad rows so pad keys self-neutralize.
  - MM1 (PE), 3 matmuls per 128-q pair instead of 4: the interior key chunk
    serves both q-tiles of the pair with one N=256 matmul. Scores land as
    [keys=128, q] in a 2-bank PSUM tile [128, 1024] per group (2 pairs).
  - ACT: ONE 1024-col exp per group (PSUM reads may span banks) -> fp16 P.
    No max-subtraction: scores ~N(0,1), exact softmax up to rounding.
  - GPSIMD: 2 strided-AP memsets per group zero the four disallowed 64x64
    corners of P post-exp in SBUF (gpsimd has no PSUM port).
  - MM2 (PE): per tile, psO[q=128, 129] accumulates P_A^T.T @ VA_A +
    P_B^T.T @ VB; col 128 (ones-column) gives the softmax denominator free.
  - DVE: copy psO -> fp16 SBUF (normalization division happens on HOST:
    out = PV/den).
  - Output written as [128, 16 pairs * 258] fp16; host divides + reassembles.

Emission is software-pipelined (group n+1's MM1 before group n's tail) and
all input DMAs are emitted first so they outrank output DMAs in scheduler
priority. PE warmup matmuls run inside the pre-data window to engage the HAM
clock gate (1.2 -> 2.4 GHz) as early as possible; everything before the flip
runs PE-throughput-bound at the cold clock.
"""

import bisect
import math

import numpy as np

B, H, S, D = 1, 16, 4096, 128
N_CORES = 8
HPC = H // N_CORES          # heads per core
TILE = 128
NT = S // TILE              # 32 query tiles per head
NPAIR = NT // 2             # 16 pairs (2 tiles each)
NGRP = NPAIR // 2           # 8 groups (2 pairs each)
PAD = 64
SPAD = S + 2 * PAD          # 4224 padded keys
NCHUNK = SPAD // TILE       # 33 key chunks
VAW = NCHUNK * (D + 1)      # 4257 cols of rearranged augmented V
SCALE = 1.0 / math.sqrt(D)

# Packed-input chunking: one chunk per group, consumption-aligned so every
# kernel slice stays inside one segment. Chunk 0 is split at pair-0
# granularity (qt 256 / kt 384 / va 258): the first MM1 only gates on the
# ~230KB half-chunk, starting compute ~1.2us sooner than the full 460KB
# chunk would.
QT_B = [0, 256] + [512 * g for g in range(1, NGRP + 1)]        # 0,256,512,...,4096
KT_B = [0, 384] + [512 * g + 640 for g in range(NGRP - 1)] + [SPAD]
VA_B = [0, 258] + [516 * g + 645 for g in range(NGRP - 1)] + [VAW]
NCK = NGRP + 1
QT_W = [QT_B[i + 1] - QT_B[i] for i in range(NCK)]
KT_W = [KT_B[i + 1] - KT_B[i] for i in range(NCK)]
VA_W = [VA_B[i + 1] - VA_B[i] for i in range(NCK)]
CHUNK_W = [QT_W[i] + KT_W[i] + VA_W[i] for i in range(NCK)]
BASE = [0]
for i in range(NCK):
    BASE.append(BASE[-1] + CHUNK_W[i])
W_PACK = BASE[-1]
OUTW = NPAIR * 258          # 4128 fp16 cols per head

_PROGRAM = None


def _qt_off(x):
    i = bisect.bisect_right(QT_B, x) - 1
    return BASE[i] + (x - QT_B[i]), i


def _kt_off(y):
    i = bisect.bisect_right(KT_B, y) - 1
    return BASE[i] + QT_W[i] + (y - KT_B[i]), i


def _va_off(z):
    i = bisect.bisect_right(VA_B, z) - 1
    return BASE[i] + QT_W[i] + KT_W[i] + (z - VA_B[i]), i


def _build_program():
    from contextlib import ExitStack

    import concourse.mybir as mybir
    import concourse.tile as tile
    from concourse import bacc

    f16 = mybir.dt.float16
    f32 = mybir.dt.float32
    Exp = mybir.ActivationFunctionType.Exp

    nc = bacc.Bacc("TRN2", target_bir_lowering=False, debug=False)
    qkv_d = nc.declare_dram_parameter("qkv", [HPC, 128, W_PACK], f16, isOutput=False)
    out_d = nc.declare_dram_parameter("out", [HPC, 128, OUTW], f16, isOutput=True)

    def qt_sl(sb, x0, w):
        off, i = _qt_off(x0)
        assert x0 + w <= QT_B[i + 1], (x0, w)
        return sb[:, off:off + w]

    def kt_sl(sb, y0, w):
        off, i = _kt_off(y0)
        assert y0 + w <= KT_B[i + 1], (y0, w)
        return sb[:, off:off + w]

    def va_sl(sb, z0, w):
        off, i = _va_off(z0)
        assert z0 + w <= VA_B[i + 1], (z0, w)
        return sb[:, off:off + w]

    with tile.TileContext(nc) as tc, ExitStack() as ctx:
        io_pool = ctx.enter_context(tc.tile_pool(name="io", bufs=2))
        out_pool = ctx.enter_context(tc.tile_pool(name="outp", bufs=2))
        # p bufs=4: with 3, the gpsimd kills(n) WAR-wait MM2(n-3) and the
        # parked wait head-of-line blocks the gpsimd queue, delaying the
        # kills and through them the MM2 LDWEIGHTS (~0.3-1.6us parks seen
        # in traces). SBUF cost of the 4th buffer is 2KB/partition.
        p_pool = ctx.enter_context(tc.tile_pool(name="p", bufs=4))
        # PSUM budget (8 banks): ps 2 bufs x 2 banks + po 2 bufs x 2 banks.
        # po=2 is the load-bearing choice: with po=1 the steady state locks
        # to a metronomic 1.50us loop [MM2span 670 + prop + CAST 694 + prop]
        # because MM2(n) WAR-waits CAST(n-1) draining the single po buffer.
        # ps=2's own loop (ACT(n) <- MM1(n) <- ps-WAR <- ACT(n-2)) is only
        # ~950ns/group at full clock with the dense 3-matmul MM1, safely
        # under the ACT engine's 1.11us/group exp throughput.
        ps_pool = ctx.enter_context(tc.tile_pool(name="ps", bufs=2, space="PSUM"))
        po_pool = ctx.enter_context(tc.tile_pool(name="po", bufs=2, space="PSUM"))

        # PE warmup: the HAM clock gate needs one ~3.4us window of sustained
        # PE activity to flip 1.2 -> 2.4 GHz. Without fillers it flips only
        # mid-kernel (measured 17-26us in) and every matmul before that runs
        # at half rate, putting PE above the chunk-arrival pace. Fill the
        # pre-data window with N=512 dummy matmuls so the flip lands at
        # ~4.5us, just as real compute ramps.
        # warm is mostly uninitialized: a full memset would delay the first
        # filler by ~1us of GPSIMD queue time; garbage operands are harmless
        # (the filler outputs are never read). The 1-element DVE memset just
        # satisfies the tile allocator's written-before-read requirement.
        warm_pool = ctx.enter_context(tc.tile_pool(name="warm", bufs=1))
        warm = warm_pool.tile([128, 512], f16, tag="warm")
        nc.vector.memset(warm[0:1, 0:1], 0.0)
        ps_warm = ps_pool.tile([128, 1024], f32, tag="ps", name="ps_warm")
        # 8x448-col fillers (~450ns issue cadence at the ~1GHz cold clock)
        # bridge the PE queue's preamble end (~7.5us) to first-chunk arrival
        # (~10.5us) with NO idle gap, so the HAM activity window that flips
        # the clock gate 1.2->2.4GHz starts counting from ~7.5us.
        for _ in range(8):
            nc.tensor.matmul(
                ps_warm[0:1, 0:448], lhsT=warm[:, 0:1], rhs=warm[:, 0:448],
                start=True, stop=True,
            )
        # Dummy exp so walrus schedules the ACT table load during the
        # pre-data window rather than before the first real activation.
        # Output goes to a separate tile so it doesn't WAR against the
        # fillers reading warm.
        dummy = warm_pool.tile([1, 16], f16, tag="dummy")
        nc.scalar.activation(dummy[0:1, 0:8], warm[0:1, 8:16], Exp, bias=0.0,
                             scale=1.0)



        # Load phase: ALL input DMAs (both heads) emitted first so they
        # outrank output DMAs in scheduler priority. Multi-wait needs on the
        # consuming matmuls are handled by framework-inserted event-
        # semaphore splits on the PE queue (~100ns each).
        io_sbs = []
        for h in range(HPC):
            io_sb = io_pool.tile([128, W_PACK], f16, tag="io")
            io_sbs.append(io_sb)
        for h in range(HPC):
            io_sb = io_sbs[h]
            for c in range(NCK):
                nc.sync.dma_start(
                    io_sb[:, BASE[c]:BASE[c + 1]], qkv_d[h, :, BASE[c]:BASE[c + 1]]
                )

        groups = [(h, g) for h in range(HPC) for g in range(NGRP)]
        out_sbs = {}
        ps_tiles = {}

        def emit_mm1(h, g):
            # Dense 3-matmul MM1 per pair (the interior key chunk serves
            # both q-tiles with one N=256 matmul). NOTE: partition-offset
            # sub-matmuls that skip the dead corners were tried and are a
            # big LOSS - writing PSUM partitions 64:128 switches the PE
            # column group (col_grp h0<->h64), ~190ns per matmul instead of
            # 56, and the HAM clock gate never engaged full rate.
            io_sb = io_sbs[h]
            ps = ps_pool.tile([128, 1024], f32, tag="ps")
            ps_tiles[(h, g)] = ps
            for j in range(2):           # pairs 2g, 2g+1
                u = 2 * g + j
                c0 = 512 * j
                # Pair u covers q-tiles 2u, 2u+1; padded key window
                # [256u, 256u+384) = key chunks u*2 .. u*2+2 at 128 stride.
                nc.tensor.matmul(
                    ps[:, c0:c0 + 128],
                    lhsT=kt_sl(io_sb, 256 * u, 128),
                    rhs=qt_sl(io_sb, 256 * u, 128), start=True, stop=True,
                )
                nc.tensor.matmul(
                    ps[:, c0 + 128:c0 + 384],
                    lhsT=kt_sl(io_sb, 256 * u + 128, 128),
                    rhs=qt_sl(io_sb, 256 * u, 256), start=True, stop=True,
                )
                nc.tensor.matmul(
                    ps[:, c0 + 384:c0 + 512],
                    lhsT=kt_sl(io_sb, 256 * u + 256, 128),
                    rhs=qt_sl(io_sb, 256 * u + 128, 128), start=True, stop=True,
                )



        def emit_tail(h, g):
            io_sb = io_sbs[h]
            out_sb = out_sbs[h]
            ps = ps_tiles.pop((h, g))
            p_sb = p_pool.tile([128, 1024], f16, tag="p")
            nc.scalar.activation(p_sb[:], ps[:], Exp, bias=0.0, scale=SCALE)
            # Kill the four disallowed 64x64 corners POST-exp on the fp16 P
            # tile in SBUF via the otherwise-idle GPSIMD engine (which has
            # no PSUM port, but P is in SBUF). Keeping kills off the DVE and
            # off the ACT-gating path leaves ACT(n) <- MM1(n) <- ACT(n-2)
            # as the only ps-WAR loop (~950ns/group at full clock). Edge
            # pads need no kill: pad kt columns are zero so scores exp to
            # exactly 1, and packed VA pad rows are all-zero INCLUDING the
            # ones-column, contributing 0 to both PV and the denominator.
            pr = p_sb.rearrange("p (a b) -> p a b", b=256)
            nc.gpsimd.memset(pr[0:64, :, 64:128], 0.0)
            nc.gpsimd.memset(pr[64:128, :, 128:192], 0.0)
            po = po_pool.tile([128, 1024], f32, tag="po")
            for j in range(2):
                u = 2 * g + j
                t0, t1 = 2 * u, 2 * u + 1
                pb = 512 * j
                ob = 512 * j          # pair j's accumulators in bank j
                nc.tensor.matmul(
                    po[:, ob:ob + 129], lhsT=p_sb[:, pb:pb + 128],
                    rhs=va_sl(io_sb, 129 * t0, 129), start=True, stop=False,
                )
                nc.tensor.matmul(
                    po[:, ob:ob + 129], lhsT=p_sb[:, pb + 128:pb + 256],
                    rhs=va_sl(io_sb, 129 * (t0 + 1), 129), start=False, stop=True,
                )
                nc.tensor.matmul(
                    po[:, ob + 129:ob + 258], lhsT=p_sb[:, pb + 256:pb + 384],
                    rhs=va_sl(io_sb, 129 * t1, 129), start=True, stop=False,
                )
                nc.tensor.matmul(
                    po[:, ob + 129:ob + 258], lhsT=p_sb[:, pb + 384:pb + 512],
                    rhs=va_sl(io_sb, 129 * (t1 + 1), 129), start=False, stop=True,
                )
            # ONE strided cast per group moves both pairs' [128,258] blocks
            # (banks 0 and 1 of po) to fp16 SBUF in a single DVE pass.
            src = po.rearrange("p (a b) -> p a b", b=512)[:, :, 0:258]
            dst = out_sb[:, 2 * g * 258:(2 * g + 2) * 258]
            nc.vector.tensor_copy(dst.rearrange("p (a b) -> p a b", b=258), src)
            # Stream output back: 4-pair chunks; the final groups go in
            # smaller chunks so the last DMA (trailing the last pair's
            # compute) is short.
            if g in (1, 3, 5):
                c0, c1 = (g - 1) * 2 * 258, (g + 1) * 2 * 258
                nc.sync.dma_start(out_d[h, :, c0:c1], out_sb[:, c0:c1])
            elif g == 6:
                c0, c1 = 12 * 258, 14 * 258
                nc.sync.dma_start(out_d[h, :, c0:c1], out_sb[:, c0:c1])
            elif g == 7:
                for u in (14, 15):
                    c0, c1 = u * 258, (u + 1) * 258
                    nc.sync.dma_start(out_d[h, :, c0:c1], out_sb[:, c0:c1])

        # DEPTH=2: PE runs MM1(g+2) while group g's ACT->memset->MM2 chain
        # completes, so the memset latency never stalls the PE stream.
        DEPTH = 2
        for n in range(len(groups) + DEPTH):
            if n < len(groups):
                h, g = groups[n]
                if g == 0:
                    out_sb = out_pool.tile([128, OUTW], f16, tag="out")
                    out_sbs[h] = out_sb
                emit_mm1(h, g)
            if n >= DEPTH:
                emit_tail(*groups[n - DEPTH])

    nc.finalize()
    return nc


def _get_program():
    global _PROGRAM
    if _PROGRAM is None:
        _PROGRAM = _build_program()
    return _PROGRAM


def _pack_inputs(q, k, v):
    """q,k,v: [H, S, D] fp32 -> packed [H, 128, W_PACK] fp16 per head."""
    qt = np.ascontiguousarray(q.transpose(0, 2, 1)).astype(np.float16)  # [H,128,S]
    k_pad = np.zeros((H, SPAD, D), np.float32)
    k_pad[:, PAD:PAD + S] = k
    kt = np.ascontiguousarray(k_pad.transpose(0, 2, 1)).astype(np.float16)
    v_aug = np.zeros((H, SPAD, D + 1), np.float32)
    v_aug[:, PAD:PAD + S, :D] = v
    # ones-column only on REAL rows: pad keys then add exp(0)*0 = 0 to both
    # PV and the denominator, so no edge-kill memsets are needed on-device.
    v_aug[:, PAD:PAD + S, D] = 1.0
    va = np.ascontiguousarray(
        v_aug.reshape(H, NCHUNK, 128, D + 1).transpose(0, 2, 1, 3)
    ).reshape(H, 128, VAW).astype(np.float16)
    segs = []
    for c in range(NCK):
        segs.append(qt[:, :, QT_B[c]:QT_B[c + 1]])
        segs.append(kt[:, :, KT_B[c]:KT_B[c + 1]])
        segs.append(va[:, :, VA_B[c]:VA_B[c + 1]])
    return np.ascontiguousarray(np.concatenate(segs, axis=2))


def kernel(q, k, v):
    """q, k, v: [1, 16, 4096, 128] float32 -> [1, 16, 4096, 128] float32."""
    from concourse.bass_utils import run_bass_kernel_spmd

    q = np.asarray(q, dtype=np.float32).reshape(H, S, D)
    k = np.asarray(k, dtype=np.float32).reshape(H, S, D)
    v = np.asarray(v, dtype=np.float32).reshape(H, S, D)

    qkv = _pack_inputs(q, k, v)
    in_maps = [
        {"qkv": np.ascontiguousarray(qkv[c * HPC:(c + 1) * HPC])}
        for c in range(N_CORES)
    ]

    nc = _get_program()
    results = run_bass_kernel_spmd(nc, in_maps, list(range(N_CORES))).results

    out = np.empty((H, S, D), np.float32)
    for c in range(N_CORES):
        o = results[c]["out"]  # [HPC, 128, 16*258] fp16, per tile [PV|den]
        for j in range(HPC):
            x = o[j].astype(np.float32).reshape(128, NT, D + 1)  # [p, t, 129]
            pv = x[:, :, :D] / x[:, :, D:D + 1]     # normalize on host
            out[c * HPC + j] = pv.transpose(1, 0, 2).reshape(S, D)
    return out.reshape(B, H, S, D)



# revision 30
# speedup vs baseline: 1.1627x; 1.0246x over previous
"""Block-sparse attention (sliding window of 3 x 64-token blocks) on 8 trn2 cores.

Problem: B=1, H=16, S=4096, D=128, fp32 I/O. Token i attends to token j iff
|i//64 - j//64| <= 1, i.e. a 192-key window per 64-query block.

Sharding: head-parallel - 2 heads per NeuronCore, no cross-core traffic.

The kernel is DMA-bandwidth-bound (per-core HBM ~358 GB/s): inputs must stay
fp16 (fp8 fails the 2e-2 error budget empirically), so per-core traffic is
6.44 MB in + 2.11 MB out (outputs in fp16, halved vs fp32) ~= 24 us at peak.
Everything else is shaped to hide under that stream:

Per-core kernel (per head = 32 q-tiles of 128, processed as 8 groups of
4 tiles / 512 queries):
  - Host packs ONE fp16 input tensor per head in consumption order: 9 chunks
    (chunk 0 split at pair granularity for a faster first MM1), each
    [qT | kT | va] where qT = Q^T [128d, S], kT = K^T padded by 64 keys
    on each end [128d, 4224], va = V augmented with a ones-column rearranged
    to [128, 33*129] (128-key chunk c at cols [129c, 129c+129)); the
    ones-column is zero on the pad rows so pad keys self-neutralize.
  - MM1 (PE), 3 matmuls per 128-q pair instead of 4: the interior key chunk
    serves both q-tiles of the pair with one N=256 matmul. Scores land as
    [keys=128, q] in a 2-bank PSUM tile [128, 1024] per group (2 pairs).
  - ACT: ONE 1024-col exp per group (PSUM reads may span banks) -> fp16 P.
    No max-subtraction: scores ~N(0,1), exact softmax up to rounding.
  - GPSIMD: 2 strided-AP memsets per group zero the four disallowed 64x64
    corners of P post-exp in SBUF (gpsimd has no PSUM port).
  - MM2 (PE): per tile, psO[q=128, 129] accumulates P_A^T.T @ VA_A +
    P_B^T.T @ VB; col 128 (ones-column) gives the softmax denominator free.
  - DVE: copy psO -> fp16 SBUF (normalization division happens on HOST:
    out = PV/den).
  - Output written as [128, 16 pairs * 258] fp16; host divides + reassembles.

Emission is software-pipelined (group n+1's MM1 before group n's tail) and
all input DMAs are emitted first so they outrank output DMAs in scheduler
priority. PE warmup matmuls run inside the pre-data window to engage the HAM
clock gate (1.2 -> 2.4 GHz) as early as possible; everything before the flip
runs PE-throughput-bound at the cold clock.
"""

import bisect
import math

import numpy as np

B, H, S, D = 1, 16, 4096, 128
N_CORES = 8
HPC = H // N_CORES          # heads per core
TILE = 128
NT = S // TILE              # 32 query tiles per head
NPAIR = NT // 2             # 16 pairs (2 tiles each)
NGRP = NPAIR // 2           # 8 groups (2 pairs each)
PAD = 64
SPAD = S + 2 * PAD          # 4224 padded keys
NCHUNK = SPAD // TILE       # 33 key chunks
VAW = NCHUNK * (D + 1)      # 4257 cols of rearranged augmented V
SCALE = 1.0 / math.sqrt(D)

# Packed-input chunking: one chunk per group, consumption-aligned so every
# kernel slice stays inside one segment. Chunk 0 is split at pair-0
# granularity (qt 256 / kt 384 / va 258): the first MM1 only gates on the
# ~230KB half-chunk, starting compute ~1.2us sooner than the full 460KB
# chunk would.
QT_B = [0, 256] + [512 * g for g in range(1, NGRP + 1)]        # 0,256,512,...,4096
KT_B = [0, 384] + [512 * g + 640 for g in range(NGRP - 1)] + [SPAD]
VA_B = [0, 258] + [516 * g + 645 for g in range(NGRP - 1)] + [VAW]
NCK = NGRP + 1
QT_W = [QT_B[i + 1] - QT_B[i] for i in range(NCK)]
KT_W = [KT_B[i + 1] - KT_B[i] for i in range(NCK)]
VA_W = [VA_B[i + 1] - VA_B[i] for i in range(NCK)]
CHUNK_W = [QT_W[i] + KT_W[i] + VA_W[i] for i in range(NCK)]
BASE = [0]
for i in range(NCK):
    BASE.append(BASE[-1] + CHUNK_W[i])
W_PACK = BASE[-1]
OUTW = NPAIR * 258          # 4128 fp16 cols per head

_PROGRAM = None


def _qt_off(x):
    i = bisect.bisect_right(QT_B, x) - 1
    return BASE[i] + (x - QT_B[i]), i


def _kt_off(y):
    i = bisect.bisect_right(KT_B, y) - 1
    return BASE[i] + QT_W[i] + (y - KT_B[i]), i


def _va_off(z):
    i = bisect.bisect_right(VA_B, z) - 1
    return BASE[i] + QT_W[i] + KT_W[i] + (z - VA_B[i]), i


def _build_program():
    from contextlib import ExitStack

    import concourse.mybir as mybir
    import concourse.tile as tile
    from concourse import bacc

    f16 = mybir.dt.float16
    f32 = mybir.dt.float32
    Exp = mybir.ActivationFunctionType.Exp

    nc = bacc.Bacc("TRN2", target_bir_lowering=False, debug=False)
    qkv_d = nc.declare_dram_parameter("qkv", [HPC, 128, W_PACK], f16, isOutput=False)
    out_d = nc.declare_dram_parameter("out", [HPC, 128, OUTW], f16, isOutput=True)

    def qt_sl(sb, x0, w):
        off, i = _qt_off(x0)
        assert x0 + w <= QT_B[i + 1], (x0, w)
        return sb[:, off:off + w]

    def kt_sl(sb, y0, w):
        off, i = _kt_off(y0)
        assert y0 + w <= KT_B[i + 1], (y0, w)
        return sb[:, off:off + w]

    def va_sl(sb, z0, w):
        off, i = _va_off(z0)
        assert z0 + w <= VA_B[i + 1], (z0, w)
        return sb[:, off:off + w]

    with tile.TileContext(nc) as tc, ExitStack() as ctx:
        io_pool = ctx.enter_context(tc.tile_pool(name="io", bufs=2))
        out_pool = ctx.enter_context(tc.tile_pool(name="outp", bufs=2))
        # p bufs=4: with 3, the gpsimd kills(n) WAR-wait MM2(n-3) and the
        # parked wait head-of-line blocks the gpsimd queue, delaying the
        # kills and through them the MM2 LDWEIGHTS (~0.3-1.6us parks seen
        # in traces). SBUF cost of the 4th buffer is 2KB/partition.
        p_pool = ctx.enter_context(tc.tile_pool(name="p", bufs=4))
        # PSUM budget (8 banks): ps 2 bufs x 2 banks + po 2 bufs x 2 banks.
        # po=2 is the load-bearing choice: with po=1 the steady state locks
        # to a metronomic 1.50us loop [MM2span 670 + prop + CAST 694 + prop]
        # because MM2(n) WAR-waits CAST(n-1) draining the single po buffer.
        # ps=2's own loop (ACT(n) <- MM1(n) <- ps-WAR <- ACT(n-2)) is only
        # ~950ns/group at full clock with the dense 3-matmul MM1, safely
        # under the ACT engine's 1.11us/group exp throughput.
        ps_pool = ctx.enter_context(tc.tile_pool(name="ps", bufs=2, space="PSUM"))
        po_pool = ctx.enter_context(tc.tile_pool(name="po", bufs=2, space="PSUM"))

        # PE warmup: the HAM clock gate needs one ~3.4us window of sustained
        # PE activity to flip 1.2 -> 2.4 GHz. Without fillers it flips only
        # mid-kernel (measured 17-26us in) and every matmul before that runs
        # at half rate, putting PE above the chunk-arrival pace. Fill the
        # pre-data window with N=512 dummy matmuls so the flip lands at
        # ~4.5us, just as real compute ramps.
        # warm is mostly uninitialized: a full memset would delay the first
        # filler by ~1us of GPSIMD queue time; garbage operands are harmless
        # (the filler outputs are never read). The 1-element DVE memset just
        # satisfies the tile allocator's written-before-read requirement.
        warm_pool = ctx.enter_context(tc.tile_pool(name="warm", bufs=1))
        warm = warm_pool.tile([128, 512], f16, tag="warm")
        nc.vector.memset(warm[0:1, 0:1], 0.0)
        ps_warm = ps_pool.tile([128, 1024], f32, tag="ps", name="ps_warm")
        # 8x448-col fillers (~450ns issue cadence at the ~1GHz cold clock)
        # bridge the PE queue's preamble end (~7.5us) to first-chunk arrival
        # (~10.5us) with NO idle gap, so the HAM activity window that flips
        # the clock gate 1.2->2.4GHz starts counting from ~7.5us.
        for _ in range(8):
            nc.tensor.matmul(
                ps_warm[0:1, 0:448], lhsT=warm[:, 0:1], rhs=warm[:, 0:448],
                start=True, stop=True,
            )
        # Dummy exp so walrus schedules the ACT table load during the
        # pre-data window rather than before the first real activation.
        # Output goes to a separate tile so it doesn't WAR against the
        # fillers reading warm.
        dummy = warm_pool.tile([1, 16], f16, tag="dummy")
        nc.scalar.activation(dummy[0:1, 0:8], warm[0:1, 8:16], Exp, bias=0.0,
                             scale=1.0)



        # Load phase: ALL input DMAs (both heads) emitted first so they
        # outrank output DMAs in scheduler priority. Multi-wait needs on the
        # consuming matmuls are handled by framework-inserted event-
        # semaphore splits on the PE queue (~100ns each).
        io_sbs = []
        for h in range(HPC):
            io_sb = io_pool.tile([128, W_PACK], f16, tag="io")
            io_sbs.append(io_sb)
        for h in range(HPC):
            io_sb = io_sbs[h]
            for c in range(NCK):
                nc.sync.dma_start(
                    io_sb[:, BASE[c]:BASE[c + 1]], qkv_d[h, :, BASE[c]:BASE[c + 1]]
                )

        groups = [(h, g) for h in range(HPC) for g in range(NGRP)]
        out_sbs = {}
        ps_tiles = {}

        def emit_mm1(h, g):
            # Dense 3-matmul MM1 per pair (the interior key chunk serves
            # both q-tiles with one N=256 matmul). NOTE: partition-offset
            # sub-matmuls that skip the dead corners were tried and are a
            # big LOSS - writing PSUM partitions 64:128 switches the PE
            # column group (col_grp h0<->h64), ~190ns per matmul instead of
            # 56, and the HAM clock gate never engaged full rate.
            io_sb = io_sbs[h]
            ps = ps_pool.tile([128, 1024], f32, tag="ps")
            ps_tiles[(h, g)] = ps
            for j in range(2):           # pairs 2g, 2g+1
                u = 2 * g + j
                c0 = 512 * j
                # Pair u covers q-tiles 2u, 2u+1; padded key window
                # [256u, 256u+384) = key chunks u*2 .. u*2+2 at 128 stride.
                nc.tensor.matmul(
                    ps[:, c0:c0 + 128],
                    lhsT=kt_sl(io_sb, 256 * u, 128),
                    rhs=qt_sl(io_sb, 256 * u, 128), start=True, stop=True,
                )
                nc.tensor.matmul(
                    ps[:, c0 + 128:c0 + 384],
                    lhsT=kt_sl(io_sb, 256 * u + 128, 128),
                    rhs=qt_sl(io_sb, 256 * u, 256), start=True, stop=True,
                )
                nc.tensor.matmul(
                    ps[:, c0 + 384:c0 + 512],
                    lhsT=kt_sl(io_sb, 256 * u + 256, 128),
                    rhs=qt_sl(io_sb, 256 * u + 128, 128), start=True, stop=True,
                )



        def emit_tail(h, g):
            io_sb = io_sbs[h]
            out_sb = out_sbs[h]
            ps = ps_tiles.pop((h, g))
            p_sb = p_pool.tile([128, 1024], f16, tag="p")
            nc.scalar.activation(p_sb[:], ps[:], Exp, bias=0.0, scale=SCALE)
            # Kill the four disallowed 64x64 corners POST-exp on the fp16 P
            # tile in SBUF via the otherwise-idle GPSIMD engine (which has
            # no PSUM port, but P is in SBUF). Keeping kills off the DVE and
            # off the ACT-gating path leaves ACT(n) <- MM1(n) <- ACT(n-2)
            # as the only ps-WAR loop (~950ns/group at full clock). Edge
            # pads need no kill: pad kt columns are zero so scores exp to
            # exactly 1, and packed VA pad rows are all-zero INCLUDING the
            # ones-column, contributing 0 to both PV and the denominator.
            pr = p_sb.rearrange("p (a b) -> p a b", b=256)
            nc.vector.memset(pr[0:64, :, 64:128], 0.0)
            nc.vector.memset(pr[64:128, :, 128:192], 0.0)
            po = po_pool.tile([128, 1024], f32, tag="po")
            for j in range(2):
                u = 2 * g + j
                t0, t1 = 2 * u, 2 * u + 1
                pb = 512 * j
                ob = 512 * j          # pair j's accumulators in bank j
                nc.tensor.matmul(
                    po[:, ob:ob + 129], lhsT=p_sb[:, pb:pb + 128],
                    rhs=va_sl(io_sb, 129 * t0, 129), start=True, stop=False,
                )
                nc.tensor.matmul(
                    po[:, ob:ob + 129], lhsT=p_sb[:, pb + 128:pb + 256],
                    rhs=va_sl(io_sb, 129 * (t0 + 1), 129), start=False, stop=True,
                )
                nc.tensor.matmul(
                    po[:, ob + 129:ob + 258], lhsT=p_sb[:, pb + 256:pb + 384],
                    rhs=va_sl(io_sb, 129 * t1, 129), start=True, stop=False,
                )
                nc.tensor.matmul(
                    po[:, ob + 129:ob + 258], lhsT=p_sb[:, pb + 384:pb + 512],
                    rhs=va_sl(io_sb, 129 * (t1 + 1), 129), start=False, stop=True,
                )
            # ONE strided cast per group moves both pairs' [128,258] blocks
            # (banks 0 and 1 of po) to fp16 SBUF in a single DVE pass.
            src = po.rearrange("p (a b) -> p a b", b=512)[:, :, 0:258]
            dst = out_sb[:, 2 * g * 258:(2 * g + 2) * 258]
            nc.vector.tensor_copy(dst.rearrange("p (a b) -> p a b", b=258), src)
            # Stream output back: 4-pair chunks; the final groups go in
            # smaller chunks so the last DMA (trailing the last pair's
            # compute) is short.
            if g in (1, 3, 5):
                c0, c1 = (g - 1) * 2 * 258, (g + 1) * 2 * 258
                nc.sync.dma_start(out_d[h, :, c0:c1], out_sb[:, c0:c1])
            elif g == 6:
                c0, c1 = 12 * 258, 14 * 258
                nc.sync.dma_start(out_d[h, :, c0:c1], out_sb[:, c0:c1])
            elif g == 7:
                for u in (14, 15):
                    c0, c1 = u * 258, (u + 1) * 258
                    nc.sync.dma_start(out_d[h, :, c0:c1], out_sb[:, c0:c1])

        # DEPTH=2: PE runs MM1(g+2) while group g's ACT->memset->MM2 chain
        # completes, so the memset latency never stalls the PE stream.
        DEPTH = 2
        for n in range(len(groups) + DEPTH):
            if n < len(groups):
                h, g = groups[n]
                if g == 0:
                    out_sb = out_pool.tile([128, OUTW], f16, tag="out")
                    out_sbs[h] = out_sb
                emit_mm1(h, g)
            if n >= DEPTH:
                emit_tail(*groups[n - DEPTH])

    nc.finalize()
    return nc


def _get_program():
    global _PROGRAM
    if _PROGRAM is None:
        _PROGRAM = _build_program()
    return _PROGRAM


def _pack_inputs(q, k, v):
    """q,k,v: [H, S, D] fp32 -> packed [H, 128, W_PACK] fp16 per head."""
    qt = np.ascontiguousarray(q.transpose(0, 2, 1)).astype(np.float16)  # [H,128,S]
    k_pad = np.zeros((H, SPAD, D), np.float32)
    k_pad[:, PAD:PAD + S] = k
    kt = np.ascontiguousarray(k_pad.transpose(0, 2, 1)).astype(np.float16)
    v_aug = np.zeros((H, SPAD, D + 1), np.float32)
    v_aug[:, PAD:PAD + S, :D] = v
    # ones-column only on REAL rows: pad keys then add exp(0)*0 = 0 to both
    # PV and the denominator, so no edge-kill memsets are needed on-device.
    v_aug[:, PAD:PAD + S, D] = 1.0
    va = np.ascontiguousarray(
        v_aug.reshape(H, NCHUNK, 128, D + 1).transpose(0, 2, 1, 3)
    ).reshape(H, 128, VAW).astype(np.float16)
    segs = []
    for c in range(NCK):
        segs.append(qt[:, :, QT_B[c]:QT_B[c + 1]])
        segs.append(kt[:, :, KT_B[c]:KT_B[c + 1]])
        segs.append(va[:, :, VA_B[c]:VA_B[c + 1]])
    return np.ascontiguousarray(np.concatenate(segs, axis=2))


def kernel(q, k, v):
    """q, k, v: [1, 16, 4096, 128] float32 -> [1, 16, 4096, 128] float32."""
    from concourse.bass_utils import run_bass_kernel_spmd

    q = np.asarray(q, dtype=np.float32).reshape(H, S, D)
    k = np.asarray(k, dtype=np.float32).reshape(H, S, D)
    v = np.asarray(v, dtype=np.float32).reshape(H, S, D)

    qkv = _pack_inputs(q, k, v)
    in_maps = [
        {"qkv": np.ascontiguousarray(qkv[c * HPC:(c + 1) * HPC])}
        for c in range(N_CORES)
    ]

    nc = _get_program()
    results = run_bass_kernel_spmd(nc, in_maps, list(range(N_CORES))).results

    out = np.empty((H, S, D), np.float32)
    for c in range(N_CORES):
        o = results[c]["out"]  # [HPC, 128, 16*258] fp16, per tile [PV|den]
        for j in range(HPC):
            x = o[j].astype(np.float32).reshape(128, NT, D + 1)  # [p, t, 129]
            pv = x[:, :, :D] / x[:, :, D:D + 1]     # normalize on host
            out[c * HPC + j] = pv.transpose(1, 0, 2).reshape(S, D)
    return out.reshape(B, H, S, D)

